# revision 11
# baseline (speedup 1.0000x reference)
"""BiLSTM-CRF Trainium2 kernel, v2 (transposed-gate layout, fp16 matmuls).

Sharding: 8 cores = 2 directions x 4 batch-groups of 8 examples (SPMD).
Per-core program:
  phase 1: P.T = Wih_perm @ x.T + b  (gate rows on partitions, token cols)
           -> DRAM pd [16, 128, NTOK] bf16, tokens stored t-major (t*bl+b)
  phase 2: LSTM scan; per step 64 Whh-stationary matmuls produce
           gates.T [128, 16gc, bl] in PSUM; P.T added via identity matmul;
           nonlinearities + cell update on 128-partition tiles; h.T written
           straight into the bf16 history (no transposes).
  phase 3: feats.T = Wo_half.T.T @ h_hist  -> featsT [16, NTOK] fp32
Host: embedding gather, gate-order permutation (i,f,o,g), time reversal for
the backward direction, summing feature halves + b_out, Viterbi DP.
"""

import numpy as np
import ml_dtypes
from contextlib import ExitStack

import concourse.bass as bass
from concourse import bacc
import concourse.mybir as mybir
from concourse import tile
from concourse.bass_utils import run_bass_kernel_spmd

F32 = mybir.dt.float32
BF16 = mybir.dt.float16
AF = mybir.ActivationFunctionType
BF = np.float16

B, S, E, H, T = 32, 512, 512, 512, 16
G4 = 4 * H            # 2048 gate rows
GC = G4 // 128        # 16 gate chunks
KE = E // 128         # 4
KH = H // 128         # 4
NCORES = 8
NGRP = 4
BL = B // NGRP        # 8 examples per core
TBLK = 64             # scan steps per P.T prefetch block


def build_program(nc, s_len=S, bl=BL, debug=False):
    ntok = s_len * bl
    # hi/lo fp16 pairs for x and Wih keep P near-fp32 exact (lo*lo dropped);
    # wo likewise.  Whh / h / P storage stay single fp16 (error ~8e-3, OK).
    xt = nc.declare_dram_parameter("xt", [2, E, ntok], BF16, isOutput=False)
    wihT = nc.declare_dram_parameter("wihT", [2, E, G4], BF16, isOutput=False)
    whhT = nc.declare_dram_parameter("whhT", [H, G4], BF16, isOutput=False)
    biasT = nc.declare_dram_parameter("biasT", [128, GC], F32, isOutput=False)
    woT = nc.declare_dram_parameter("woT", [H, 2 * T], BF16, isOutput=False)
    ident = nc.declare_dram_parameter("ident", [128, 128], BF16, isOutput=False)
    featsT = nc.declare_dram_parameter("featsT", [T, ntok], F32, isOutput=True)
    pd = nc.dram_tensor("pscr", [GC, 128, ntok], BF16)
    if debug:
        pdump = nc.declare_dram_parameter("pdump", [GC, 128, ntok], BF16, isOutput=True)
        hdump = nc.declare_dram_parameter(
            "hdump", [128, (s_len + 1) * KH * bl], BF16, isOutput=True)

    n_mt = ntok // 512
    nblk = s_len // TBLK

    with tile.TileContext(nc) as tc, ExitStack() as ctx:
        wpool = ctx.enter_context(tc.tile_pool(name="persist", bufs=1))
        whh_sb = wpool.tile([128, KH, G4], BF16, tag="whh")
        nc.sync.dma_start(whh_sb[:], whhT.rearrange("(k p) n -> p k n", p=128))
        wo_sb = wpool.tile([128, KH, 2 * T], BF16, tag="wo")
        nc.sync.dma_start(wo_sb[:], woT.rearrange("(k p) n -> p k n", p=128))
        id_sb = wpool.tile([128, 128], BF16, tag="id")
        nc.sync.dma_start(id_sb[:], ident[:])
        bias_sb = wpool.tile([128, GC], F32, tag="bias")
        nc.sync.dma_start(bias_sb[:], biasT[:])
        # h.T history: slot 0 = h_init = 0; step t reads slot t, writes t+1
        hist = wpool.tile([128, s_len + 1, KH, bl], BF16, tag="hist")
        c_t = wpool.tile([128, KH, bl], F32, tag="c")
        nc.gpsimd.memset(hist[:, 0, :, :], 0.0)
        nc.gpsimd.memset(c_t[:], 0.0)

        # ---- phase 1: P.T = Wih_perm @ x.T + b ----
        with tc.tile_pool(name="xtl", bufs=3) as xp, \
             tc.tile_pool(name="p1ps", bufs=4, space="PSUM") as pp, \
             tc.tile_pool(name="wihp", bufs=1) as wihp, \
             tc.tile_pool(name="pout", bufs=4) as pop:
            wih_sb = wihp.tile([128, 2, KE, G4], BF16)
            nc.sync.dma_start(wih_sb[:], wihT.rearrange("h (k p) n -> p h k n", p=128))
            xtr = xt.rearrange("h (k p) n -> p h k n", p=128)
            for mt in range(n_mt):
                xt_sb = xp.tile([128, 2, KE, 512], BF16)
                nc.sync.dma_start(xt_sb[:], xtr[:, :, :, mt * 512:(mt + 1) * 512])
                for gc in range(GC):
                    ps = pp.tile([128, 512], F32)
                    # P = Wh*xh + Wh*xl + Wl*xh  (lo*lo term negligible)
                    terms = [(0, 0), (0, 1), (1, 0)]
                    for ti, (wi, xi) in enumerate(terms):
                        for k in range(KE):
                            nc.tensor.matmul(
                                ps[:], wih_sb[:, wi, k, gc * 128:(gc + 1) * 128],
                                xt_sb[:, xi, k, :],
                                start=(ti == 0 and k == 0),
                                stop=(ti == len(terms) - 1 and k == KE - 1))
                    po = pop.tile([128, 512], BF16)
                    nc.scalar.activation(po[:], ps[:], AF.Identity,
                                         bias=bias_sb[:, gc:gc + 1])
                    nc.sync.dma_start(pd[gc, :, mt * 512:(mt + 1) * 512], po[:])

        # ---- phase 2: scan ----
        pdr = pd.rearrange("g p n -> p g n")
        with tc.tile_pool(name="ptb", bufs=2) as ptp, \
             tc.tile_pool(name="g2ps", bufs=3, space="PSUM") as gpsp, \
             tc.tile_pool(name="acts", bufs=4) as ap:
            for blk in range(nblk):
                cols = TBLK * bl
                ptb = ptp.tile([128, GC, cols], BF16, tag="pt")
                nc.sync.dma_start(
                    ptb[:], pdr[:, :, blk * cols:(blk + 1) * cols])
                for tb in range(TBLK):
                    t = blk * TBLK + tb
                    ps2 = gpsp.tile([128, GC, bl], F32, tag="gate")
                    # P.T into PSUM first (identity matmul), gates accumulate
                    nc.tensor.matmul(
                        ps2[:, :, :], id_sb[:],
                        ptb[:, :, tb * bl:(tb + 1) * bl],
                        start=True, stop=False, skip_group_check=True)
                    # g-gates (12..15) first so the ACT/DVE tail overlaps the
                    # remaining matmuls; o-gates (8..11) last (needed last).
                    for gc in [12, 13, 14, 15] + list(range(12)):
                        for k in range(KH):
                            nc.tensor.matmul(
                                ps2[:, gc, :],
                                whh_sb[:, k, gc * 128:(gc + 1) * 128],
                                hist[:, t, k, :],
                                start=False, stop=(k == KH - 1),
                                skip_group_check=True)
                    a_g = ap.tile([128, KH, bl], F32, tag="tg")
                    nc.scalar.activation(a_g[:], ps2[:, 12:16, :], AF.Tanh)
                    a_if = ap.tile([128, 8, bl], F32, tag="sif")
                    nc.scalar.activation(a_if[:], ps2[:, 0:8, :], AF.Sigmoid)
                    a_o = ap.tile([128, KH, bl], F32, tag="so")
                    nc.scalar.activation(a_o[:], ps2[:, 8:12, :], AF.Sigmoid)
                    t1 = ap.tile([128, KH, bl], F32, tag="t1")
                    nc.vector.tensor_mul(t1[:], a_if[:, 0:4, :], a_g[:])
                    nc.gpsimd.tensor_mul(c_t[:], c_t[:], a_if[:, 4:8, :])
                    nc.vector.tensor_add(c_t[:], c_t[:], t1[:])
                    a_tc = ap.tile([128, KH, bl], F32, tag="tc")
                    nc.scalar.activation(a_tc[:], c_t[:], AF.Tanh)
                    nc.vector.tensor_mul(hist[:, t + 1, :, :],
                                         a_o[:], a_tc[:])

        if debug:
            nc.sync.dma_start(
                hdump.rearrange("p (t k b) -> p t k b", k=KH, b=bl), hist[:])
            with tc.tile_pool(name="dbg", bufs=2) as dbp:
                for gc in range(GC):
                    db = dbp.tile([128, ntok], BF16)
                    nc.sync.dma_start(db[:], pd[gc])
                    nc.sync.dma_start(pdump[gc], db[:])

        # ---- phase 3: feats.T = woT.T @ hist ----
        with tc.tile_pool(name="f3", bufs=2) as f3p, \
             tc.tile_pool(name="f3ps", bufs=2, space="PSUM") as f3ps:
            for mt in range(n_mt):
                t0 = mt * (512 // bl)
                ps3 = f3ps.tile([T, 512], F32)
                for hl in range(2):
                    for k in range(KH):
                        nc.tensor.matmul(
                            ps3[:], wo_sb[:, k, hl * T:(hl + 1) * T],
                            hist[:, 1 + t0:1 + t0 + 512 // bl, k, :],
                            start=(hl == 0 and k == 0),
                            stop=(hl == 1 and k == KH - 1))
                fo = f3p.tile([T, 512], F32)
                nc.vector.tensor_copy(fo[:], ps3[:])
                nc.sync.dma_start(featsT[:, mt * 512:(mt + 1) * 512], fo[:])
    return nc


_NC_CACHE = {}


def _get_nc(s_len=S, debug=False):
    key = ("nc", s_len, debug)
    if key not in _NC_CACHE:
        nc = bacc.Bacc("TRN2")
        build_program(nc, s_len=s_len, debug=debug)
        nc.finalize()
        _NC_CACHE[key] = nc
    return _NC_CACHE[key]


def _perm_gates(w):
    """Permute PyTorch gate order i,f,g,o -> i,f,o,g along axis 0."""
    i, f, g, o = w[0:H], w[H:2 * H], w[2 * H:3 * H], w[3 * H:4 * H]
    return np.concatenate([i, f, o, g], axis=0)


def make_in_maps(emb, Wih_f, Whh_f, b_f, Wih_b, Whh_b, b_b, W_out, s_len=S, bl=BL):
    """emb: [B, s_len, E] float32. Returns 8 per-core input maps."""
    ident = np.eye(128, dtype=BF)

    def hilo(a):
        hi = a.astype(BF)
        lo = (a - hi.astype(np.float32)).astype(BF)
        return np.stack([hi, lo])

    prepped = {}
    for d in range(2):
        Wih, Whh, bvec = (Wih_f, Whh_f, b_f) if d == 0 else (Wih_b, Whh_b, b_b)
        Wih = _perm_gates(np.asarray(Wih, np.float32))
        Whh = _perm_gates(np.asarray(Whh, np.float32))
        bvec = _perm_gates(np.asarray(bvec, np.float32)[:, None])[:, 0]
        wo_half = np.asarray(W_out, np.float32)[:, :H] if d == 0 \
            else np.asarray(W_out, np.float32)[:, H:]
        wo_hl = hilo(np.ascontiguousarray(wo_half.T))          # [2, H, T]
        prepped[d] = {
            "wihT": hilo(np.ascontiguousarray(Wih.T)),         # [2, E, G4]
            "whhT": np.ascontiguousarray(Whh.T).astype(BF),
            "biasT": np.ascontiguousarray(bvec.reshape(GC, 128).T).astype(np.float32),
            "woT": np.ascontiguousarray(
                np.concatenate([wo_hl[0], wo_hl[1]], axis=1)),  # [H, 2T]
            "ident": ident,
        }
    in_maps = []
    for c in range(NCORES):
        d, g = divmod(c, NGRP)
        x = emb[g * bl:(g + 1) * bl]
        if d == 1:
            x = x[:, ::-1]
        # token-major cols: col = t*bl + b  ->  [E, s_len*bl]
        xtm = np.ascontiguousarray(
            x.transpose(2, 1, 0).reshape(E, s_len * bl)).astype(np.float32)
        in_maps.append({"xt": hilo(xtm), **prepped[d]})
    return in_maps


def assemble_feats(results, b_out, s_len=S, bl=BL):
    feats = np.zeros((NGRP * bl, s_len, T), np.float32)
    for c in range(NCORES):
        d, g = divmod(c, NGRP)
        f = np.asarray(results[c]["featsT"], np.float32)  # [T, s_len*bl]
        f = f.reshape(T, s_len, bl).transpose(2, 1, 0)    # [bl, s, T]
        if d == 1:
            f = f[:, ::-1]
        feats[g * bl:(g + 1) * bl] += f
    feats += np.asarray(b_out, np.float32)[None, None, :]
    return feats


def viterbi(feats, trans, start, stop):
    Bq, Sq, Tq = feats.shape
    v = feats[:, 0] + start[None, :]
    idxs = np.zeros((Sq - 1, Bq, Tq), np.int32)
    for s in range(1, Sq):
        scores = v[:, :, None] + trans[None, :, :]
        idxs[s - 1] = np.argmax(scores, axis=1)
        v = np.max(scores, axis=1) + feats[:, s]
    last = np.argmax(v + stop[None, :], axis=-1).astype(np.int32)
    tags = np.zeros((Bq, Sq), np.int32)
    tags[:, -1] = last
    cur = last
    for s in range(Sq - 2, -1, -1):
        cur = idxs[s][np.arange(Bq), cur].astype(np.int32)
        tags[:, s] = cur
    return tags


def kernel(sentence, embedding, Wih_f, Whh_f, b_f, Wih_b, Whh_b, b_b,
           W_out, b_out, transitions, start_trans, stop_trans):
    sentence = np.asarray(sentence)
    emb = np.asarray(embedding, np.float32)[sentence.astype(np.int64)]  # [B, S, E]
    nc = _get_nc()
    in_maps = make_in_maps(emb, Wih_f, Whh_f, b_f, Wih_b, Whh_b, b_b, W_out)
    res = run_bass_kernel_spmd(nc, in_maps, list(range(NCORES))).results
    feats = assemble_feats(res, np.asarray(b_out))
    return viterbi(feats, np.asarray(transitions, np.float32),
                   np.asarray(start_trans, np.float32),
                   np.asarray(stop_trans, np.float32))


# revision 13
# speedup vs baseline: 1.0966x; 1.0966x over previous
"""BiLSTM-CRF Trainium2 kernel (transposed-gate layout, fp16 + hi/lo splits).

Sharding: 8 cores = 2 directions x 4 batch-groups of 8 examples (SPMD).
Per-core program:
  phase 1: P.T = Wih_perm @ x.T + b  (gate rows on partitions, token cols)
           x and Wih as hi/lo fp16 pairs (3 accumulating matmul passes)
           -> DRAM pd [16, 128, NTOK] fp16, tokens stored t-major (t*bl+b)
  phase 2: LSTM scan; per step 64 Whh-stationary matmuls produce
           gates.T [128, 16gc, bl] in PSUM; P.T added via identity matmul;
           nonlinearities + cell update on 128-partition tiles; h.T written
           straight into the fp16 history (no transposes).
  phase 3: feats.T = Wo_half(hi/lo).T.T @ h_hist  -> featsT [16, NTOK] fp32
Host: embedding gather, gate-order permutation (i,f,o,g), time reversal for
the backward direction, summing feature halves + b_out, Viterbi DP.
"""

import numpy as np
import ml_dtypes
from contextlib import ExitStack

import concourse.bass as bass
from concourse import bacc
import concourse.mybir as mybir
from concourse import tile
from concourse.bass_utils import run_bass_kernel_spmd

F32 = mybir.dt.float32
BF16 = mybir.dt.float16
AF = mybir.ActivationFunctionType
BF = np.float16

B, S, E, H, T = 32, 512, 512, 512, 16
G4 = 4 * H            # 2048 gate rows
GC = G4 // 128        # 16 gate chunks
KE = E // 128         # 4
KH = H // 128         # 4
NCORES = 8
NGRP = 4
BL = B // NGRP        # 8 examples per core
TBLK = 64             # scan steps per P.T prefetch block


def build_program(nc, s_len=S, bl=BL, debug=False):
    ntok = s_len * bl
    # hi/lo fp16 pairs for x and Wih keep P near-fp32 exact (lo*lo dropped);
    # wo likewise.  Whh / h / P storage stay single fp16 (error ~8e-3, OK).
    xt = nc.declare_dram_parameter("xt", [2, E, ntok], BF16, isOutput=False)
    wihT = nc.declare_dram_parameter("wihT", [2, E, G4], BF16, isOutput=False)
    whhT = nc.declare_dram_parameter("whhT", [H, G4], BF16, isOutput=False)
    biasT = nc.declare_dram_parameter("biasT", [128, GC], F32, isOutput=False)
    woT = nc.declare_dram_parameter("woT", [H, 2 * T], BF16, isOutput=False)
    ident = nc.declare_dram_parameter("ident", [128, 128], BF16, isOutput=False)
    featsT = nc.declare_dram_parameter("featsT", [T, ntok], F32, isOutput=True)
    pd = nc.dram_tensor("pscr", [GC, 128, ntok], BF16)
    if debug:
        pdump = nc.declare_dram_parameter("pdump", [GC, 128, ntok], BF16, isOutput=True)
        hdump = nc.declare_dram_parameter(
            "hdump", [128, (s_len + 1) * KH * bl], BF16, isOutput=True)

    n_mt = ntok // 512
    nblk = s_len // TBLK

    with tile.TileContext(nc) as tc, ExitStack() as ctx:
        wpool = ctx.enter_context(tc.tile_pool(name="persist", bufs=1))
        whh_sb = wpool.tile([128, KH, G4], BF16, tag="whh")
        nc.sync.dma_start(whh_sb[:], whhT.rearrange("(k p) n -> p k n", p=128))
        wo_sb = wpool.tile([128, KH, 2 * T], BF16, tag="wo")
        nc.sync.dma_start(wo_sb[:], woT.rearrange("(k p) n -> p k n", p=128))
        id_sb = wpool.tile([128, 128], BF16, tag="id")
        nc.sync.dma_start(id_sb[:], ident[:])
        bias_sb = wpool.tile([128, GC], F32, tag="bias")
        nc.sync.dma_start(bias_sb[:], biasT[:])
        wih_sb = wpool.tile([128, 2, KE, G4], BF16, tag="wih")
        nc.sync.dma_start(wih_sb[:], wihT.rearrange("h (k p) n -> p h k n", p=128))
        # h.T history: slot 0 = h_init = 0; step t reads slot t, writes t+1
        hist = wpool.tile([128, s_len + 1, KH, bl], BF16, tag="hist")
        c_t = wpool.tile([128, KH, bl], F32, tag="c")
        nc.gpsimd.memset(hist[:, 0, :, :], 0.0)
        nc.gpsimd.memset(c_t[:], 0.0)

        xtr = xt.rearrange("h (k p) n -> p h k n", p=128)
        pdr = pd.rearrange("g p n -> p g n")

        # phase 1 (P.T = Wih_perm @ x.T + b) is emitted interleaved with the
        # scan: tile mt is produced while scan block mt-2 runs, one gc-group
        # every 4 steps, so its matmuls fill the scan's h-wait stalls.
        with tc.tile_pool(name="xtl", bufs=2) as xp, \
             tc.tile_pool(name="p1ps", bufs=2, space="PSUM") as pp, \
             tc.tile_pool(name="pout", bufs=4) as pop, \
             tc.tile_pool(name="ptb", bufs=2) as ptp, \
             tc.tile_pool(name="g2ps", bufs=3, space="PSUM") as gpsp, \
             tc.tile_pool(name="acts", bufs=4) as ap:

            def p1_fetch(mt):
                xt_sb = xp.tile([128, 2, KE, 512], BF16, tag="xt")
                nc.sync.dma_start(xt_sb[:], xtr[:, :, :, mt * 512:(mt + 1) * 512])
                return xt_sb

            def p1_gc(mt, gc, xt_sb):
                ps = pp.tile([128, 512], F32, tag="p1")
                # P = Wh*xh + Wh*xl + Wl*xh  (lo*lo term negligible)
                terms = [(0, 0), (0, 1), (1, 0)]
                for ti, (wi, xi) in enumerate(terms):
                    for k in range(KE):
                        nc.tensor.matmul(
                            ps[:], wih_sb[:, wi, k, gc * 128:(gc + 1) * 128],
                            xt_sb[:, xi, k, :],
                            start=(ti == 0 and k == 0),
                            stop=(ti == len(terms) - 1 and k == KE - 1))
                po = pop.tile([128, 512], BF16, tag="po")
                nc.scalar.activation(po[:], ps[:], AF.Identity,
                                     bias=bias_sb[:, gc:gc + 1])
                nc.sync.dma_start(pd[gc, :, mt * 512:(mt + 1) * 512], po[:])

            for mt in range(min(2, n_mt)):
                xt_sb = p1_fetch(mt)
                for gc in range(GC):
                    p1_gc(mt, gc, xt_sb)

            for blk in range(nblk):
                cols = TBLK * bl
                ptb = ptp.tile([128, GC, cols], BF16, tag="pt")
                nc.sync.dma_start(
                    ptb[:], pdr[:, :, blk * cols:(blk + 1) * cols])
                nxt = blk + 2
                xt_nxt = p1_fetch(nxt) if nxt < n_mt else None
                for tb in range(TBLK):
                    t = blk * TBLK + tb
                    ps2 = gpsp.tile([128, GC, bl], F32, tag="gate")
                    # P.T into PSUM first (identity matmul), gates accumulate
                    nc.tensor.matmul(
                        ps2[:, :, :], id_sb[:],
                        ptb[:, :, tb * bl:(tb + 1) * bl],
                        start=True, stop=False, skip_group_check=True)
                    # g-gates (12..15) first so the ACT/DVE tail overlaps the
                    # remaining matmuls; o-gates (8..11) last (needed last).
                    for gc in [12, 13, 14, 15] + list(range(12)):
                        for k in range(KH):
                            nc.tensor.matmul(
                                ps2[:, gc, :],
                                whh_sb[:, k, gc * 128:(gc + 1) * 128],
                                hist[:, t, k, :],
                                start=False, stop=(k == KH - 1),
                                skip_group_check=True)
                    a_g = ap.tile([128, KH, bl], F32, tag="tg")
                    nc.scalar.activation(a_g[:], ps2[:, 12:16, :], AF.Tanh)
                    a_if = ap.tile([128, 8, bl], F32, tag="sif")
                    nc.scalar.activation(a_if[:], ps2[:, 0:8, :], AF.Sigmoid)
                    a_o = ap.tile([128, KH, bl], F32, tag="so")
                    nc.scalar.activation(a_o[:], ps2[:, 8:12, :], AF.Sigmoid)
                    t1 = ap.tile([128, KH, bl], F32, tag="t1")
                    nc.vector.tensor_mul(t1[:], a_if[:, 0:4, :], a_g[:])
                    nc.gpsimd.tensor_mul(c_t[:], c_t[:], a_if[:, 4:8, :])
                    nc.vector.tensor_add(c_t[:], c_t[:], t1[:])
                    a_tc = ap.tile([128, KH, bl], F32, tag="tc")
                    nc.scalar.activation(a_tc[:], c_t[:], AF.Tanh)
                    nc.vector.tensor_mul(hist[:, t + 1, :, :],
                                         a_o[:], a_tc[:])
                    if xt_nxt is not None and tb % 4 == 0:
                        p1_gc(nxt, tb // 4, xt_nxt)

        if debug:
            nc.sync.dma_start(
                hdump.rearrange("p (t k b) -> p t k b", k=KH, b=bl), hist[:])
            with tc.tile_pool(name="dbg", bufs=2) as dbp:
                for gc in range(GC):
                    db = dbp.tile([128, ntok], BF16)
                    nc.sync.dma_start(db[:], pd[gc])
                    nc.sync.dma_start(pdump[gc], db[:])

        # ---- phase 3: feats.T = woT.T @ hist ----
        with tc.tile_pool(name="f3", bufs=2) as f3p, \
             tc.tile_pool(name="f3ps", bufs=2, space="PSUM") as f3ps:
            for mt in range(n_mt):
                t0 = mt * (512 // bl)
                ps3 = f3ps.tile([T, 512], F32)
                for hl in range(2):
                    for k in range(KH):
                        nc.tensor.matmul(
                            ps3[:], wo_sb[:, k, hl * T:(hl + 1) * T],
                            hist[:, 1 + t0:1 + t0 + 512 // bl, k, :],
                            start=(hl == 0 and k == 0),
                            stop=(hl == 1 and k == KH - 1))
                fo = f3p.tile([T, 512], F32)
                nc.vector.tensor_copy(fo[:], ps3[:])
                nc.sync.dma_start(featsT[:, mt * 512:(mt + 1) * 512], fo[:])
    return nc


_NC_CACHE = {}


def _get_nc(s_len=S, debug=False):
    key = ("nc", s_len, debug)
    if key not in _NC_CACHE:
        nc = bacc.Bacc("TRN2")
        build_program(nc, s_len=s_len, debug=debug)
        nc.finalize()
        _NC_CACHE[key] = nc
    return _NC_CACHE[key]


def _perm_gates(w):
    """Permute PyTorch gate order i,f,g,o -> i,f,o,g along axis 0."""
    i, f, g, o = w[0:H], w[H:2 * H], w[2 * H:3 * H], w[3 * H:4 * H]
    return np.concatenate([i, f, o, g], axis=0)


def make_in_maps(emb, Wih_f, Whh_f, b_f, Wih_b, Whh_b, b_b, W_out, s_len=S, bl=BL):
    """emb: [B, s_len, E] float32. Returns 8 per-core input maps."""
    ident = np.eye(128, dtype=BF)

    def hilo(a):
        hi = a.astype(BF)
        lo = (a - hi.astype(np.float32)).astype(BF)
        return np.stack([hi, lo])

    prepped = {}
    for d in range(2):
        Wih, Whh, bvec = (Wih_f, Whh_f, b_f) if d == 0 else (Wih_b, Whh_b, b_b)
        Wih = _perm_gates(np.asarray(Wih, np.float32))
        Whh = _perm_gates(np.asarray(Whh, np.float32))
        bvec = _perm_gates(np.asarray(bvec, np.float32)[:, None])[:, 0]
        wo_half = np.asarray(W_out, np.float32)[:, :H] if d == 0 \
            else np.asarray(W_out, np.float32)[:, H:]
        wo_hl = hilo(np.ascontiguousarray(wo_half.T))          # [2, H, T]
        prepped[d] = {
            "wihT": hilo(np.ascontiguousarray(Wih.T)),         # [2, E, G4]
            "whhT": np.ascontiguousarray(Whh.T).astype(BF),
            "biasT": np.ascontiguousarray(bvec.reshape(GC, 128).T).astype(np.float32),
            "woT": np.ascontiguousarray(
                np.concatenate([wo_hl[0], wo_hl[1]], axis=1)),  # [H, 2T]
            "ident": ident,
        }
    in_maps = []
    for c in range(NCORES):
        d, g = divmod(c, NGRP)
        x = emb[g * bl:(g + 1) * bl]
        if d == 1:
            x = x[:, ::-1]
        # token-major cols: col = t*bl + b  ->  [E, s_len*bl]
        xtm = np.ascontiguousarray(
            x.transpose(2, 1, 0).reshape(E, s_len * bl)).astype(np.float32)
        in_maps.append({"xt": hilo(xtm), **prepped[d]})
    return in_maps


def assemble_feats(results, b_out, s_len=S, bl=BL):
    feats = np.zeros((NGRP * bl, s_len, T), np.float32)
    for c in range(NCORES):
        d, g = divmod(c, NGRP)
        f = np.asarray(results[c]["featsT"], np.float32)  # [T, s_len*bl]
        f = f.reshape(T, s_len, bl).transpose(2, 1, 0)    # [bl, s, T]
        if d == 1:
            f = f[:, ::-1]
        feats[g * bl:(g + 1) * bl] += f
    feats += np.asarray(b_out, np.float32)[None, None, :]
    return feats


def viterbi(feats, trans, start, stop):
    Bq, Sq, Tq = feats.shape
    v = feats[:, 0] + start[None, :]
    idxs = np.zeros((Sq - 1, Bq, Tq), np.int32)
    for s in range(1, Sq):
        scores = v[:, :, None] + trans[None, :, :]
        idxs[s - 1] = np.argmax(scores, axis=1)
        v = np.max(scores, axis=1) + feats[:, s]
    last = np.argmax(v + stop[None, :], axis=-1).astype(np.int32)
    tags = np.zeros((Bq, Sq), np.int32)
    tags[:, -1] = last
    cur = last
    for s in range(Sq - 2, -1, -1):
        cur = idxs[s][np.arange(Bq), cur].astype(np.int32)
        tags[:, s] = cur
    return tags


def kernel(sentence, embedding, Wih_f, Whh_f, b_f, Wih_b, Whh_b, b_b,
           W_out, b_out, transitions, start_trans, stop_trans):
    sentence = np.asarray(sentence)
    emb = np.asarray(embedding, np.float32)[sentence.astype(np.int64)]  # [B, S, E]
    nc = _get_nc()
    in_maps = make_in_maps(emb, Wih_f, Whh_f, b_f, Wih_b, Whh_b, b_b, W_out)
    res = run_bass_kernel_spmd(nc, in_maps, list(range(NCORES))).results
    feats = assemble_feats(res, np.asarray(b_out))
    return viterbi(feats, np.asarray(transitions, np.float32),
                   np.asarray(start_trans, np.float32),
                   np.asarray(stop_trans, np.float32))
